# revision 52
# baseline (speedup 1.0000x reference)
"""DualSlidingWindowAttention Trainium2 kernel.

Sharding: 8 cores = 2 batches x 4 head-groups. Core (b, m) owns batch b,
q-heads 8m..8m+7, kv-heads 2m, 2m+1. Host sums the 4 partial o-proj outputs
per batch.

Per-core device program (single TileContext, engines overlap aggressively):
  Warm-up: 32 dummy ident matmuls ramp the HAM to full clock while the
           first xt tiles stream in.
  Phase A: kv projections, paced by the xt DMA stream (sync queue) while
           weights stream on the scalar queue. v is PE-transposed to
           [token, D] layout for the o matmuls; a 64-token-shifted copy of
           the ssm v (sbuf->sbuf DMAs, free) serves the ssm subtile path.
  Phase B1: q projection for the second token half (qtiles 4-7), c-pairs
           interleaved across two psum banks (same-bank back-to-back
           accumulation serializes drains at ~+65%/MM). The first-half q
           projection is interleaved into stretch 1 below.
  Phase C: block-sparse attention. Per (kv-group, 128-query tile) the
           attn window needs up to 3 wide 128-kv chunks (N=512, 4 heads
           interleaved); the 64-wide ssm window is processed as two
           64-query subtiles, each against one 64-aligned 128-kv window
           (N=256) - half the score/exp/o work of the naive 2-chunk form.
           Softmax: exp(s/8) on ACT, mask*exp(alibi) multiply on DVE,
           sums via a ones-column appended to v (free on the PE).
           Normalization is inline per unit: the sums row is DMA'd into a
           [2, 256] staging tile, reciprocal'd on DVE (f16), broadcast
           across partitions by a K=2 selector matmul, applied by one DVE
           multiply. The recip runs one unit ahead of the broadcast so
           the in-order PE never waits.
           Stretch 1 = qtiles 4-7 with the 128 first-half q-proj MMs as
           PE filler; stretch 2 = qtiles 0-3 (tiny qt0 units last) with
           the hh=1 o-proj as filler; the hh=0 o-proj tail follows, its
           evacuation copies alternating DVE/ACT and its output DMAs
           rotating over three queues.

All matmul operands are fp16 (FWL weight loads, half-sized DMA);
accumulation is fp32 in PSUM; softmax sums stay fp32, reciprocals f16.
"""

import sys

sys.path.insert(0, "/opt/trn_rl_repo")

import numpy as np
import concourse.bass as bass
import concourse.bacc as bacc
import concourse.mybir as mybir
import concourse.tile as tile

F32 = mybir.dt.float32
F16 = mybir.dt.float16

HID, H, HK, G, D, T = 2048, 32, 8, 4, 64, 1024
W_ATT, W_SSM = 256, 64
NQT = T // 128  # 8 query tiles
KVG = 2         # kv heads (= head groups) per core
HL = 4          # q heads per kv group

# slot order: [attn_left, ssm_left, attn_full, attn_causal, ssm_causal]
SLOT_SRC = [1, 0, 1, 1, 0]       # 1 = hidden (attn window), 0 = ssm
SLOT_CHOFF = [-2, -1, -1, 0, 0]  # kv chunk offset relative to qtile
SLOT_OFF = [-256, -128, -128, 0, 0]
SLOT_WIN = [W_ATT, W_SSM, W_ATT, W_ATT, W_SSM]


def first_slot(qt):
    return {0: 3, 1: 1}.get(qt, 0)


def build_program():
    nc = bacc.Bacc("TRN2", target_bir_lowering=False, debug=False)

    xt_ssm = nc.declare_dram_parameter("xt_ssm", [HID, T], F16, isOutput=False)
    xt_hid = nc.declare_dram_parameter("xt_hid", [HID, T], F16, isOutput=False)
    wq = nc.declare_dram_parameter("wq", [128, 4, 32, 128], F16, isOutput=False)
    wk = nc.declare_dram_parameter("wk", [128, 16, 128], F16, isOutput=False)
    wv = nc.declare_dram_parameter("wv", [128, 16, 128], F16, isOutput=False)
    wsk = nc.declare_dram_parameter("wsk", [128, 16, 128], F16, isOutput=False)
    wsv = nc.declare_dram_parameter("wsv", [128, 16, 128], F16, isOutput=False)
    wo = nc.declare_dram_parameter("wo", [128, 4, 2048], F16, isOutput=False)
    mconc = nc.declare_dram_parameter("mconc", [128, 10, 512], F16, isOutput=False)
    ident = nc.declare_dram_parameter("ident", [128, 128], F16, isOutput=False)
    sel = nc.declare_dram_parameter("sel", [2, 128], F16, isOutput=False)
    out_t = nc.declare_dram_parameter("out_t", [HID, T], F16, isOutput=True)

    mm = nc.tensor.matmul

    with tile.TileContext(nc) as tc:
        with tc.tile_pool(name="persist", bufs=1) as pers:
            # ---- persistent sbuf tiles (live for the whole kernel) ----
            xt_sb = {}
            for src in range(2):
                for kc in range(16):
                    xt_sb[(src, kc)] = pers.tile(
                        [128, T], F16, tag=f"xt{src}_{kc}", name=f"xt{src}_{kc}")
            qT_sb = pers.tile([128, NQT, HL * 128], F16, tag="qT")
            kT_sb = [pers.tile([128, T], F16, tag=f"kT{s}", name=f"kT{s}")
                     for s in range(2)]
            # v_sb[src][kvh]: [tok-in-chunk, chunk, D+1]; col 64 = ones
            v_sb = [
                [pers.tile([128, NQT, 65], F16, tag=f"v{s}{h}", name=f"v{s}{h}")
                 for h in range(2)]
                for s in range(2)
            ]
            # vsh_sb[kvh]: ssm v shifted by 64 tokens -- partition p of
            # chunk c holds token c*128 + p - 64, so a 64-aligned 128-token
            # window (the ssm subtile window) is a plain partition slice.
            vsh_sb = [pers.tile([128, NQT, 65], F16, tag=f"vsh{h}",
                                name=f"vsh{h}") for h in range(2)]
            ident_sb = pers.tile([128, 128], F16, tag="ident")
            sel_sb = pers.tile([2, 128], F16, tag="sel")
            oT_sb = pers.tile([128, 4, T], F32, tag="oT")
            oTb_sb = pers.tile([128, 4, T], F16, tag="oTb")
            m_sb = pers.tile([128, 10, 512], F16, tag="mconc")
            wo_sb = pers.tile([128, 4, 2048], F16, tag="wo")

            # ---------------- Phases A + B1: projections ----------------
            with tc.tile_pool(name="wqp", bufs=1) as wqp:
                wq_sb = [wqp.tile([128, 32, 128], F16, tag=f"wq{c}", name=f"wq{c}")
                         for c in range(4)]
                with tc.tile_pool(name="w4p", bufs=1) as w4p:
                    w4_t = {"wsk": wsk, "wsv": wsv, "wk": wk, "wv": wv}
                    w4_sb = {}
                    for name in ("wsk", "wsv", "wk", "wv"):
                        w4_sb[name] = w4p.tile([128, 16, 128], F16, tag=name,
                                               name=name)
                    stage_sb = [w4p.tile([128, T], F16, tag=f"stg{s}",
                                         name=f"stg{s}") for s in range(2)]

                    # Small weights on the scalar queue; the fat transfers
                    # all on the sync queue ORDERED xt -> wq -> wo, so the
                    # kv phase (paced by xt) gets the full HBM bandwidth
                    # and wq tiles land just in time for the B1 c-loop.
                    nc.scalar.dma_start(out=ident_sb, in_=ident[:, :])
                    for name in ("wsk", "wsv", "wk", "wv"):
                        nc.scalar.dma_start(out=w4_sb[name], in_=w4_t[name][:, :, :])
                    nc.scalar.dma_start(out=sel_sb, in_=sel[:, :])
                    for vsrc in range(2):
                        for vh in range(2):
                            nc.vector.memset(v_sb[vsrc][vh][:, :, 64:65], 1.0)
                    for vh in range(2):
                        nc.vector.memset(vsh_sb[vh][:, :, 64:65], 1.0)
                    nc.scalar.dma_start(out=m_sb, in_=mconc[:, :, :])

                    for src, xt_t in ((0, xt_ssm), (1, xt_hid)):
                        for kc in range(16):
                            nc.sync.dma_start(
                                out=xt_sb[(src, kc)],
                                in_=xt_t[kc * 128:(kc + 1) * 128, :])
                    for c in range(4):
                        nc.sync.dma_start(out=wq_sb[c][:, :, :], in_=wq[:, c, :, :])
                    nc.sync.dma_start(out=wo_sb, in_=wo[:, :, :])

                    # PE warm-up: dummy matmuls on ident while the first xt
                    # tile and wsk stream in. The HAM reaches full clock
                    # only after ~3us of continuous PE activity; without
                    # this the first ~15us of kv projections run at half
                    # rate.
                    with tc.tile_pool(name="wup", bufs=2, space="PSUM") as wup:
                        for i in range(32):
                            wt = wup.tile([128, 128], F32, tag="wu",
                                          name=f"wu{i}")
                            mm(wt[:, :], lhsT=ident_sb[:, :],
                               rhs=ident_sb[:, :], start=True, stop=True)

                    # -- Phase A: kv projections, 4 psum groups per src --
                    with (
                        tc.tile_pool(name="kvp", bufs=3, space="PSUM") as kvp,
                        tc.tile_pool(name="tp", bufs=2, space="PSUM") as tp,
                    ):
                        for src in range(2):
                            wk_t = w4_sb["wk" if src else "wsk"]
                            wv_t = w4_sb["wv" if src else "wsv"]
                            kps = [kvp.tile([128, 512], F32, tag="kps",
                                            name=f"kps{src}_{h}") for h in range(2)]
                            vps = [kvp.tile([128, 512], F32, tag="vps",
                                            name=f"vps{src}_{h}") for h in range(2)]
                            for kc in range(16):
                                xtile = xt_sb[(src, kc)]
                                for h in range(2):
                                    mm(kps[h][:, :], lhsT=wk_t[:, kc, :],
                                       rhs=xtile[:, h * 512:(h + 1) * 512],
                                       start=(kc == 0), stop=(kc == 15))
                                    mm(vps[h][:, :], lhsT=wv_t[:, kc, :],
                                       rhs=xtile[:, h * 512:(h + 1) * 512],
                                       start=(kc == 0), stop=(kc == 15))
                            for h in range(2):
                                nc.vector.tensor_copy(
                                    kT_sb[src][:, h * 512:(h + 1) * 512],
                                    kps[h][:, :])
                                nc.vector.tensor_copy(
                                    stage_sb[src][:, h * 512:(h + 1) * 512],
                                    vps[h][:, :])
                        # transposes after both srcs' matmuls so the PE never
                        # waits on the DVE stage evacuations mid-stream; the
                        # psum->sbuf copies alternate ACT/DVE so the 4-deep
                        # tp rotation keeps the PE streaming.
                        for src in range(2):
                            for h in range(2):
                                for j in range(8):
                                    tp_t = tp.tile([128, 64], F16, tag="tp")
                                    nc.tensor.transpose(
                                        tp_t[:, :],
                                        stage_sb[src][h * 64:(h + 1) * 64,
                                                      j * 128:(j + 1) * 128],
                                        ident_sb[h * 64:(h + 1) * 64,
                                                 h * 64:(h + 1) * 64])
                                    if j % 2 == 0:
                                        nc.scalar.copy(
                                            v_sb[src][h][:, j, 0:64],
                                            tp_t[:, :])
                                    else:
                                        nc.vector.tensor_copy(
                                            v_sb[src][h][:, j, 0:64],
                                            tp_t[:, :])


                # shifted ssm v copies (sbuf->sbuf DMAs, partition shift by
                # 64): free on the engines, overlapped with the B1 matmuls.
                for h in range(2):
                    for j in range(8):
                        nc.sync.dma_start(
                            out=vsh_sb[h][64:128, j, 0:64],
                            in_=v_sb[0][h][0:64, j, 0:64])
                        if j < 7:
                            nc.gpsimd.dma_start(
                                out=vsh_sb[h][0:64, j + 1, 0:64],
                                in_=v_sb[0][h][64:128, j, 0:64])

                # -- Phase B1: q projection, second token half (qt 4-7) --
                # c-pairs interleave across two psum banks so consecutive
                # accumulating matmuls never drain into the same bank
                # (same-bank back-to-back accumulation costs ~+65% per MM).
                with tc.tile_pool(name="qp1", bufs=4, space="PSUM") as qp1:
                    for cp in range(2):
                        qpair = [qp1.tile([128, 512], F32, tag="qps",
                                          name=f"qps1_{2 * cp + i}")
                                 for i in range(2)]
                        for src in range(2):
                            for kc in range(16):
                                for i in range(2):
                                    mm(qpair[i][:, :],
                                       lhsT=wq_sb[2 * cp + i][:, src * 16 + kc, :],
                                       rhs=xt_sb[(src, kc)][:, 512:1024],
                                       start=(src == 0 and kc == 0),
                                       stop=(src == 1 and kc == 15))
                        # host permutes Wq cols so col-tile c =
                        # [head c (kvg0), head 4+c (kvg1)].
                        for i in range(2):
                            c = 2 * cp + i
                            nc.vector.tensor_copy(
                                qT_sb[:, 4:8, c * 128:(c + 1) * 128],
                                qpair[i][:, :].rearrange(
                                    "p (qt j) -> p qt j", j=128))

                # ---------------- Phase C: attention ----------------
                # qt 4-7 first (their q is ready); the first-half q-proj
                # MMs fill every PE gap in that stretch. qt 0-3 follow,
                # with the second-half o-proj as the filler. kvg-major so
                # a half-stretch norm (4 units) completes a whole kvg and
                # unblocks the o-proj c-columns that read it.
                # stretch 2 ends with the two tiny qt0 units so the exposed
                # final chain (exp -> o -> norm -> tail) is as short as
                # possible.
                units = [(kvg, qt) for kvg in range(KVG)
                         for qt in (4, 5, 6, 7)]
                units += [(0, 1), (0, 2), (0, 3), (1, 1), (1, 2), (1, 3),
                          (0, 0), (1, 0)]

                with (
                    tc.tile_pool(name="weip", bufs=4) as weip,
                    tc.tile_pool(name="ostgp", bufs=2) as ostgp,
                    tc.tile_pool(name="outstgp", bufs=4) as outstgp,
                    tc.tile_pool(name="rrp", bufs=4) as rrp,
                ):
                    wei_tiles = {}
                    norm_st = {}
                    pools = {}

                    def slot_range(qt):
                        # wei/m_sb slots used by this qtile: qt0 keeps the
                        # wide legacy pair {3,4}; qt>=1 use wide attn slots
                        # {[0,]2,3} plus the 64-query ssm subtile block in
                        # slot 1.
                        return (3, 5) if qt == 0 else (1, 4) if qt == 1 else (0, 4)

                    def wide_slots(qt):
                        return (3, 4) if qt == 0 else (2, 3) if qt == 1 \
                            else (0, 2, 3)

                    def emit_scores(u, filler=None):
                        kvg, qt = units[u]
                        wei_t = weip.tile([128, 5, 512], F16, tag="wei")
                        wei_tiles[u] = wei_t
                        for s in wide_slots(qt):
                            ch = qt + SLOT_CHOFF[s]
                            sp_t = pools["sp"].tile([128, 512], F32, tag="sp")
                            mm(sp_t[:, :],
                               lhsT=kT_sb[SLOT_SRC[s]][kvg * 64:(kvg + 1) * 64,
                                                       ch * 128:(ch + 1) * 128],
                               rhs=qT_sb[kvg * 64:(kvg + 1) * 64, qt, :],
                               start=True, stop=True)
                            nc.scalar.activation(
                                out=wei_t[:, s, :], in_=sp_t[:, :],
                                func=mybir.ActivationFunctionType.Exp,
                                scale=0.125)
                        # sp has 3 bufs; the filler sits before the 4th
                        # allocation so the first slot's bank has drained
                        # through the exp.
                        if filler is not None:
                            filler()
                        if qt >= 1:
                            # ssm subtile scores: per 64-query subtile one
                            # misaligned-but-64-aligned 128-kv window.
                            sp_t = pools["sp"].tile([128, 512], F32, tag="sp")
                            qT4 = qT_sb[kvg * 64:(kvg + 1) * 64, qt,
                                        :].rearrange("p (h q) -> p h q", q=128)
                            for sub in range(2):
                                kv0 = qt * 128 + sub * 64 - 64
                                mm(sp_t[:, sub * 256:(sub + 1) * 256],
                                   lhsT=kT_sb[0][kvg * 64:(kvg + 1) * 64,
                                                 kv0:kv0 + 128],
                                   rhs=qT4[:, :, sub * 64:(sub + 1) * 64],
                                   start=True, stop=True)
                            nc.scalar.activation(
                                out=wei_t[:, 1, :], in_=sp_t[:, :],
                                func=mybir.ActivationFunctionType.Exp,
                                scale=0.125)
                        # DVE only: GPSIMD takes ~4.5us for this multiply and
                        # stalls the in-order PE stream at the o matmuls.
                        lo, hi = slot_range(qt)
                        nc.vector.tensor_mul(
                            wei_t[:, lo:hi, :], wei_t[:, lo:hi, :],
                            m_sb[:, kvg * 5 + lo:kvg * 5 + hi, :])

                    def emit_o(u):
                        kvg, qt = units[u]
                        wei_t = wei_tiles.pop(u)
                        op_t = pools["op"].tile([128, 512], F32, tag="op")
                        ws = wide_slots(qt)
                        for i, s in enumerate(ws):
                            ch = qt + SLOT_CHOFF[s]
                            mm(op_t[0:65, :],
                               lhsT=v_sb[SLOT_SRC[s]][kvg][:, ch, :],
                               rhs=wei_t[:, s, :],
                               start=(i == 0),
                               stop=(qt == 0 and s == 4))
                        if qt >= 1:
                            op4 = op_t[0:65, :].rearrange(
                                "p (h q) -> p h q", q=128)
                            for sub in range(2):
                                vt = vsh_sb[kvg] if sub == 0 else v_sb[0][kvg]
                                mm(op4[:, :, sub * 64:(sub + 1) * 64],
                                   lhsT=vt[:, qt, :],
                                   rhs=wei_t[:, 1,
                                             sub * 256:(sub + 1) * 256],
                                   start=False, stop=(sub == 1))
                        # scatter unnormalized o into oT and the softmax-sum
                        # row into a [par, (t, j)] staging tile for this
                        # unit's inline normalization. Engines cannot cross
                        # partitions, so stage in SBUF and scatter with DMAs.
                        ostg = ostgp.tile([128, 512], F32, tag="ostg")
                        # ACT: DVE is the attention stretches' busiest
                        # engine (wei-mask mul, norm muls, outstg copies).
                        nc.scalar.copy(ostg[0:65, :], op_t[0:65, :])
                        src4 = ostg[:, :].rearrange("p (t pr j) -> p t pr j",
                                                    t=2, pr=2)
                        for par in range(2):
                            eng = nc.sync if par == 0 else nc.gpsimd
                            eng.dma_start(
                                out=oT_sb[par * 64:(par + 1) * 64,
                                          kvg * 2:kvg * 2 + 2,
                                          qt * 128:(qt + 1) * 128],
                                in_=src4[0:64, :, par, :])
                        rr = rrp.tile([2, 256], F32, tag="rr")
                        norm_st[u] = rr
                        for par in range(2):
                            # gpsimd: the lightest DMA queue during the
                            # attention stretches.
                            nc.gpsimd.dma_start(
                                out=rr[par:par + 1, :].rearrange(
                                    "pr (t j) -> pr t j", t=2),
                                in_=src4[64:65, :, par, :])

                    # inline per-unit normalization, split in two so the
                    # K=2 broadcast matmul never waits in the in-order PE
                    # stream: the reciprocal runs a unit earlier on DVE.
                    def emit_recip(u):
                        rr = norm_st.pop(u)
                        rrb = rrp.tile([2, 256], F16, tag="rrb")
                        with nc.allow_low_precision(
                                reason="f16 softmax normalizer: <=2^-11 "
                                       "relative error, well inside budget"):
                            nc.vector.reciprocal(rrb[:, :], rr[:, :])
                        norm_st[u] = rrb

                    def emit_normbc(u):
                        kvg, qt = units[u]
                        rrb = norm_st.pop(u)
                        bc_t = pools["bcp"].tile([128, 256], F32, tag="bc")
                        mm(bc_t[:, :], lhsT=sel_sb[:, :],
                           rhs=rrb[0:2, :].rearrange("pr (t j) -> pr t j",
                                                     t=2),
                           start=True, stop=True)
                        nc.vector.tensor_mul(
                            oTb_sb[:, kvg * 2:kvg * 2 + 2,
                                   qt * 128:(qt + 1) * 128],
                            oT_sb[:, kvg * 2:kvg * 2 + 2,
                                  qt * 128:(qt + 1) * 128],
                            bc_t[:, :].rearrange("p (t j) -> p t j", j=128))

                    def emit_oproj(hh, ns, pool, tag):
                        for n in ns:
                            p3_t = pool.tile([128, 512], F32, tag=tag)
                            for c in range(4):
                                mm(p3_t[:, :],
                                   lhsT=wo_sb[:, c, n * 128:(n + 1) * 128],
                                   rhs=oTb_sb[:, c, hh * 512:(hh + 1) * 512],
                                   start=(c == 0), stop=(c == 3))
                            outstg = outstgp.tile([128, 512], F16, tag="outstg")
                            # hh=0 is the tail: ACT is idle there, so
                            # alternating engines halves the copy drain.
                            if hh == 0 and n % 2 == 1:
                                nc.scalar.copy(outstg[:, :], p3_t[:, :])
                            else:
                                nc.vector.tensor_copy(outstg[:, :], p3_t[:, :])
                            eng = (nc.sync, nc.gpsimd, nc.scalar)[n % 3]
                            eng.dma_start(
                                out=out_t[n * 128:(n + 1) * 128,
                                          hh * 512:(hh + 1) * 512],
                                in_=outstg[:, :])

                    with (
                        tc.tile_pool(name="sp", bufs=3, space="PSUM") as sp_,
                        tc.tile_pool(name="op", bufs=2, space="PSUM") as op_,
                        tc.tile_pool(name="bcp", bufs=1, space="PSUM") as bcp_,
                    ):
                        pools["sp"], pools["op"] = sp_, op_
                        pools["bcp"] = bcp_

                        # -- Stretch 1: units 0-7 (qt 4-7) + first-half q --
                        with tc.tile_pool(name="qp0", bufs=2,
                                          space="PSUM") as qp0:
                            qp0_ps = {}

                            def h0_mm(c, src, kc):
                                if c not in qp0_ps:
                                    qp0_ps[c] = qp0.tile(
                                        [128, 512], F32, tag="qps0",
                                        name=f"qps0_{c}")
                                mm(qp0_ps[c][:, :],
                                   lhsT=wq_sb[c][:, src * 16 + kc, :],
                                   rhs=xt_sb[(src, kc)][:, 0:512],
                                   start=(src == 0 and kc == 0),
                                   stop=(src == 1 and kc == 15))

                            def h0_evac(c):
                                nc.vector.tensor_copy(
                                    qT_sb[:, 0:4, c * 128:(c + 1) * 128],
                                    qp0_ps.pop(c)[:, :].rearrange(
                                        "p (qt j) -> p qt j", j=128))

                            # c-pairs alternate banks (same-bank drain
                            # serialization, as in B1).
                            h0_ops = []
                            for cp in range(2):
                                for src in range(2):
                                    for kc in range(16):
                                        for i in range(2):
                                            h0_ops.append(
                                                lambda c=2 * cp + i, src=src,
                                                kc=kc: h0_mm(c, src, kc))
                                h0_ops.append(
                                    lambda c=2 * cp: h0_evac(c))
                                h0_ops.append(
                                    lambda c=2 * cp + 1: h0_evac(c))

                            def pop_h0(k):
                                for _ in range(k):
                                    if h0_ops:
                                        h0_ops.pop(0)()

                            for u in range(8):
                                emit_scores(u, filler=lambda: pop_h0(5))
                                pop_h0(4)
                                if u >= 2:
                                    emit_o(u - 2)
                                    pop_h0(4)
                                if u >= 3:
                                    emit_recip(u - 3)
                                if u >= 4:
                                    emit_normbc(u - 4)
                                pop_h0(4)
                            pop_h0(len(h0_ops))
                            emit_o(6)
                            emit_recip(5)
                            emit_normbc(4)
                            emit_o(7)
                            emit_recip(6)
                            emit_normbc(5)
                            emit_recip(7)
                            emit_normbc(6)
                            emit_normbc(7)

                        # -- Stretch 2: units 8-15 (qt 0-3) + o-proj hh=1 --
                        # -- Stretch 2: units 8-15 (qt 0-3) + o-proj hh=1 --
                        # units 0-7 are all normalized by now, so the hh=1
                        # o-proj can fill from the first unit.
                        with tc.tile_pool(name="oprojp", bufs=2,
                                          space="PSUM") as oprojp:
                            op_ns = list(range(16))

                            def pop_oproj(k):
                                ns, op_ns[:] = op_ns[:k], op_ns[k:]
                                if ns:
                                    emit_oproj(1, ns, oprojp, "oprojp")

                            for u in range(8, 16):
                                emit_scores(u, filler=lambda: pop_oproj(1))
                                if u < 14:
                                    pop_oproj(1)
                                if u >= 10:
                                    emit_o(u - 2)
                                if u >= 11:
                                    emit_recip(u - 3)
                                if u >= 12:
                                    emit_normbc(u - 4)
                            emit_o(14)
                            emit_recip(13)
                            emit_normbc(12)
                            pop_oproj(1)
                            emit_o(15)
                            emit_recip(14)
                            emit_normbc(13)
                            pop_oproj(16)
                            emit_recip(15)
                            emit_normbc(14)
                            emit_normbc(15)
                    # tail: the unit-region psum pools are closed; a 4-deep
                    # pool lets the last 64 oproj matmuls stream without
                    # copy stalls.
                    with tc.tile_pool(name="tailp", bufs=4,
                                      space="PSUM") as tailp:
                        emit_oproj(0, range(16), tailp, "tailp")

    nc.finalize()
    return nc


def make_mconc(m):
    """Mask*exp(alibi) tile for core head-group m: [128, 10, 512] f16.

    Slots 0/2/3/4 are the wide (128-query) blocks as in the reference slot
    decomposition. Slot 1 holds the 64-query ssm subtile block twice
    (sub-major): for a query subtile starting at q0, the kv window is
    [q0-64, q0+64) so rel = kv - q = (p - 64) - j for p in [0,128),
    j in [0,64); valid iff 0 <= -rel < 64. Identical for every (qt, sub).
    """
    p = np.arange(128)[:, None]
    j = np.arange(128)[None, :]
    out = np.zeros((128, 10, 512), np.float16)
    for kvg in range(KVG):
        for s in (0, 2, 3, 4):
            rel = SLOT_OFF[s] + p - j  # [128, 128] kv - q
            mask = (-rel >= 0) & (-rel < SLOT_WIN[s])
            for hl in range(HL):
                hg = 8 * m + kvg * 4 + hl
                slope = 2.0 ** (-8.0 * hg / H)
                vals = np.where(mask, np.exp(slope * rel.astype(np.float64)), 0.0)
                out[:, kvg * 5 + s, hl * 128:(hl + 1) * 128] = vals.astype(np.float16)
        js = np.arange(64)[None, :]
        rel_s = (p - 64) - js  # [128, 64] kv - q within the ssm subtile
        mask_s = (-rel_s >= 0) & (-rel_s < W_SSM)
        blk = np.zeros((128, 256), np.float16)
        for hl in range(HL):
            hg = 8 * m + kvg * 4 + hl
            slope = 2.0 ** (-8.0 * hg / H)
            vals = np.where(mask_s, np.exp(slope * rel_s.astype(np.float64)), 0.0)
            blk[:, hl * 64:(hl + 1) * 64] = vals.astype(np.float16)
        out[:, kvg * 5 + 1, 0:256] = blk
        out[:, kvg * 5 + 1, 256:512] = blk
    return out


def make_inputs(core, hidden_states, ssm_states, Wq, Wk, Wv, Wsk, Wsv, Wo):
    b, m = core // 4, core % 4
    f16 = lambda x: np.ascontiguousarray(np.asarray(x, dtype=np.float16))

    def wshard(W, cols, nchunk):
        # [K, cols] -> [128, K//128, cols]
        Ws = np.asarray(W)[:, cols]
        return f16(Ws.reshape(nchunk, 128, Ws.shape[1]).transpose(1, 0, 2))

    # col-tile c = [head c (kvg0) cols, head 4+c (kvg1) cols]
    qperm = np.concatenate(
        [np.arange(64) + 64 * h for c in range(4) for h in (c, 4 + c)])
    qcols = 512 * m + qperm
    kvcols = slice(128 * m, 128 * (m + 1))
    wq_sh = wshard(Wq, qcols, 32)                      # [128, 32, 512]
    wq_sh = np.ascontiguousarray(
        wq_sh.reshape(128, 32, 4, 128).transpose(0, 2, 1, 3))  # c-major
    wo_sh = np.asarray(Wo)[512 * m:512 * (m + 1), :]
    sel = np.zeros((2, 128), np.float16)
    sel[0, 0:64] = 1.0
    sel[1, 64:128] = 1.0
    return {
        "xt_ssm": f16(np.asarray(ssm_states)[b].T),
        "xt_hid": f16(np.asarray(hidden_states)[b].T),
        "wq": wq_sh,
        "wk": wshard(Wk, kvcols, 16),
        "wv": wshard(Wv, kvcols, 16),
        "wsk": wshard(Wsk, kvcols, 16),
        "wsv": wshard(Wsv, kvcols, 16),
        "wo": f16(wo_sh.reshape(4, 128, 2048).transpose(1, 0, 2)),
        "mconc": make_mconc(m),
        "ident": np.eye(128, dtype=np.float16),
        "sel": sel,
    }


def gather(results):
    out = np.zeros((2, T, HID), np.float32)
    for core in range(8):
        b = core // 4
        out[b] += results[core]["out_t"].T.astype(np.float32)
    return out


# ----------------------------------------------------------------------------
# Harness entry point
# ----------------------------------------------------------------------------
_NC_CACHE = []


def _get_program():
    if not _NC_CACHE:
        _NC_CACHE.append(build_program())
    return _NC_CACHE[0]


def _run(inp, trace=False):
    from concourse.bass_utils import run_bass_kernel_spmd

    nc = _get_program()
    in_maps = [make_inputs(core, **{k: np.asarray(inp[k]) for k in (
        "hidden_states", "ssm_states", "Wq", "Wk", "Wv", "Wsk", "Wsv", "Wo")})
        for core in range(8)]
    res = run_bass_kernel_spmd(nc, in_maps, list(range(8)), trace=trace)
    return gather(res.results), res.exec_time_ns


def kernel(hidden_states, ssm_states, Wq, Wk, Wv, Wsk, Wsv, Wo):
    out, _ = _run(dict(
        hidden_states=hidden_states, ssm_states=ssm_states, Wq=Wq, Wk=Wk,
        Wv=Wv, Wsk=Wsk, Wsv=Wsv, Wo=Wo))
    return out
